# revision 34
# baseline (speedup 1.0000x reference)
"""Multi-head attention (B=2, L=S=2048, D=1024, H=16) on 8 Trainium2 cores.

Sharding: core c -> batch b = c // 4, head group g = c % 4 (4 heads per core).
W_Q/K/V column-sharded (256 cols per core), W_O row-sharded (256 rows per core);
the 4 partial outputs per batch are summed on the host (plus bias terms).

Per-core pipeline (all big tensors kept transposed so no on-device transposes):
  projections: QT = 0.125*(x Wq + bq)^T, KT = (x Wk)^T (feature-major
    [256, L]; bk is dropped - (q+bq)@(k+bk) differs from (q+bq)@k only by a
    per-row constant, which softmax cancels); Vaug = [V_h | ones] per head
    (seq-major, fp16), V bias folded out on the host (softmax rows sum to 1
    => + bv @ Wo + bo once).
  attention, per (l-tile 512, s-tile 128): S^T = KT^T QT (row-packed pairs of
    heads, K=64); E = exp(S^T) * maskT (ACT exp from PSUM, 0/1 fp16 mask
    multiply on DVE); T_h += Vaug_h^T E accumulates BOTH the head output AND
    its softmax row-sums in one full-array matmul.
  out-projection: out_partial = outT^T Wo_rows (K=128, accumulate over the
    two 128-row groups).

Scheduling: the K/V/Q projections are emitted just-in-time inside the
attention st-loop (s-chunk pipelined), the AV matmuls lag the score matmuls
by two s-tiles, and each l-tile's output projection is spread through the
next l-tile's loop - so the PE never drains and the ACT engine starts
exp'ing early. Bulk loads are spread across the two hardware DGE rings
(SP + Activation) and the per-l-tile mask loads are split into quarters so
no single transfer blocks the rings.
"""
from contextlib import ExitStack

import numpy as np

import concourse.bass as bass
import concourse.mybir as mybir
import concourse.tile as tile
from concourse import bacc
from concourse.bass_utils import run_bass_kernel_spmd

F16 = mybir.dt.float16
F32 = mybir.dt.float32

D = 1024          # d_model
H = 16            # heads
DK = 64           # head dim
B, L = 2, 2048
NCORES = 8
HPC = 4           # heads per core
FPC = HPC * DK    # features per core = 256
KD = D // 128     # 8 contraction subtiles for projections
LT, LTW = 4, 512  # l tiles
ST, STW = 16, 128  # s tiles
Exp = mybir.ActivationFunctionType.Exp
Mult = mybir.AluOpType.mult
Add = mybir.AluOpType.add

_CACHED_NC = None


def _build():
    nc = bacc.Bacc("TRN2", target_bir_lowering=False, debug=False,
                   num_devices=NCORES)
    xT = nc.declare_dram_parameter("xT", [LT, 128, KD, LTW], F16,
                                   isOutput=False)
    wq = nc.declare_dram_parameter("wq", [128, KD, FPC], F16, isOutput=False)
    wk = nc.declare_dram_parameter("wk", [128, KD, FPC], F16, isOutput=False)
    wv = nc.declare_dram_parameter("wv", [128, KD, FPC], F16, isOutput=False)
    wo = nc.declare_dram_parameter("wo", [128, 2, D], F16, isOutput=False)
    bq = nc.declare_dram_parameter("bq", [128, 2], F32, isOutput=False)
    maskT = nc.declare_dram_parameter("maskT", [LT, 128, ST, LTW], F16,
                                      isOutput=False)
    out = nc.declare_dram_parameter("out", [128, ST, D], F16, isOutput=True)

    with tile.TileContext(nc) as tc, ExitStack() as ctx:
        pool = ctx.enter_context(tc.tile_pool(name="pers", bufs=1))
        mpool = ctx.enter_context(tc.tile_pool(name="mpool", bufs=3))
        epool = ctx.enter_context(tc.tile_pool(name="epool", bufs=4))
        rbpool = ctx.enter_context(tc.tile_pool(name="rbpool", bufs=6))
        opool = ctx.enter_context(tc.tile_pool(name="opool", bufs=4))
        scp = ctx.enter_context(tc.tile_pool(name="scp", bufs=2, space="PSUM"))
        tp = ctx.enter_context(tc.tile_pool(name="tp", bufs=1, space="PSUM"))

        xt = pool.tile([128, LT, KD, LTW], F16)   # s-chunk major
        wq_sb = pool.tile([128, KD, FPC], F16)
        wk_sb = pool.tile([128, KD, FPC], F16)
        wv_sb = pool.tile([128, KD, FPC], F16)
        wo_sb = pool.tile([128, 2, D], F16)
        bq_sb = pool.tile([128, 2], F32)
        mks = [None, None, None]  # mpool mask tiles, [128, ST, LTW]

        # Bulk loads split across the SP and ACT hardware-DGE rings; x chunk
        # 0 is halved so the first KT matmuls start as early as possible.
        nc.scalar.dma_start(out=wk_sb[:], in_=wk[:])
        nc.sync.dma_start(out=xt[:, 0, 0:4], in_=xT[0, :, 0:4])
        nc.sync.dma_start(out=xt[:, 0, 4:KD], in_=xT[0, :, 4:KD])
        nc.scalar.dma_start(out=wv_sb[:], in_=wv[:])
        nc.sync.dma_start(out=wq_sb[:], in_=wq[:])
        nc.sync.dma_start(out=bq_sb[:], in_=bq[:])
        mks[0] = mpool.tile([128, ST, LTW], F16, tag="mk", name="mk0")
        for j in range(4):
            nc.sync.dma_start(out=mks[0][:, 4 * j:4 * j + 4, :],
                              in_=maskT[0, :, 4 * j:4 * j + 4, :])
        nc.sync.dma_start(out=xt[:, 1], in_=xT[1])
        nc.sync.dma_start(out=xt[:, 2], in_=xT[2])
        nc.sync.dma_start(out=xt[:, 3], in_=xT[3])
        nc.sync.dma_start(out=wo_sb[:], in_=wo[:])

        QT = pool.tile([128, 2, L], F16)   # [feat(2x128), l]: Q^T * 0.125
        KT = pool.tile([128, 2, L], F16)
        # Vaug[:, st, h]: even h -> [V_h | 1], odd h -> [1 | V_h]
        Vaug = pool.tile([128, ST, HPC, 128], F16)
        nc.gpsimd.memset(Vaug[:], 1.0)
        outTs = [pool.tile([128, 2, LTW], F16, name=f"outT{i}")
                 for i in range(LT)]
        Ts = [None] * HPC  # per-lt PSUM accumulators, rebound each lt

        # ---------------- JIT emission helpers ----------------
        kt_ps = [None]

        def emit_kt_half(c, ft, half):
            fsl = slice(ft * 128, (ft + 1) * 128)
            if half == 0:
                kt_ps[0] = scp.tile([128, 2, LTW], F32, tag="sc",
                                    name=f"pk{c}_{ft}")
            ps = kt_ps[0]
            for kd in range(4 * half, 4 * half + 4):
                nc.tensor.matmul(ps[:, 0, :], wk_sb[:, kd, fsl],
                                 xt[:, c, kd, :],
                                 start=(kd == 0), stop=(kd == KD - 1))
            if half == 1:
                nc.vector.tensor_copy(KT[:, ft, c * LTW:(c + 1) * LTW],
                                      ps[:, 0, :])

        qt_ps = [None]

        def emit_qt_half(lt, ft, half):
            fsl = slice(ft * 128, (ft + 1) * 128)
            if half == 0:
                qt_ps[0] = scp.tile([128, 2, LTW], F32, tag="sc",
                                    name=f"pq{lt}_{ft}")
            ps = qt_ps[0]
            for kd in range(4 * half, 4 * half + 4):
                nc.tensor.matmul(ps[:, 0, :], wq_sb[:, kd, fsl],
                                 xt[:, lt, kd, :],
                                 start=(kd == 0), stop=(kd == KD - 1))
            if half == 1:
                nc.vector.scalar_tensor_tensor(
                    QT[:, ft, lt * LTW:(lt + 1) * LTW], ps[:, 0, :], 0.125,
                    bq_sb[:, ft:ft + 1].to_broadcast((128, LTW)),
                    Mult, Add)

        def emit_v_st(st):
            c, r = divmod(st, 4)
            ssl = slice(r * STW, (r + 1) * STW)
            psv = scp.tile([128, 2, LTW], F32, tag="sc", name=f"psv{st}")
            for kd in range(KD):
                nc.tensor.matmul(psv[:, 0, :FPC], xt[:, c, kd, ssl],
                                 wv_sb[:, kd, :],
                                 start=(kd == 0), stop=(kd == KD - 1))
            # two strided copies cover all four heads: even-h V lands at
            # lane 0:64 of its slot, odd-h V at 64:128.
            vg = Vaug[:, st].rearrange("p (a c) d -> p a (c d)", a=2, c=2)
            pv = psv[:, 0, 0:FPC].rearrange("p (a cd) -> p a cd", a=2)
            nc.vector.tensor_copy(vg[:, :, 0:64], pv[:, :, 0:64])
            nc.vector.tensor_copy(vg[:, :, 192:256], pv[:, :, 64:128])

        def emit_outproj_tile(lt8):
            ps3 = scp.tile([128, 2, LTW], F32, tag="sc", name=f"ps3_{lt8}")
            for nf in range(2):
                nsl = slice(nf * 512, (nf + 1) * 512)
                for pair in range(2):
                    nc.tensor.matmul(
                        ps3[:, nf, :],
                        outTs[lt8 // 4][:, pair,
                                        (lt8 % 4) * 128:(lt8 % 4 + 1) * 128],
                        wo_sb[:, pair, nsl],
                        start=(pair == 0), stop=(pair == 1))
            ob = opool.tile([128, D], F16)
            nc.vector.tensor_copy(ob[:], ps3[:])
            eng = nc.gpsimd if lt8 % 2 == 0 else nc.sync
            eng.dma_start(out=out[:, lt8, :], in_=ob[:])

        def emit_normalize_h(lt, h):
            # reciprocal_approx_fast only works at partition base 0, so
            # route the row sums through lanes 0:64 in both parities.
            pair, i = divmod(h, 2)
            av_sl = slice(64 * i, 64 * (i + 1))        # av lanes
            rs_sl = slice(64 * (1 - i), 64 * (2 - i))  # row-sum lanes
            rb = rbpool.tile([128, LTW], F32)
            if i == 0:   # av 0:64, sums 64:128 -> move sums down first
                nc.vector.tensor_copy(rb[64:128, :], Ts[h][rs_sl, :])
                nc.gpsimd.dma_start(out=rb[0:64, :], in_=rb[64:128, :])
                nc.vector.reciprocal_approx_fast(out=rb[0:64, :],
                                                 in_=rb[0:64, :])
            else:        # sums 0:64 -> recip at base 0, then move up
                nc.vector.reciprocal_approx_fast(out=rb[0:64, :],
                                                 in_=Ts[h][rs_sl, :])
                nc.gpsimd.dma_start(out=rb[64:128, :], in_=rb[0:64, :])
            nc.vector.tensor_mul(outTs[lt][av_sl, pair, :],
                                 Ts[h][av_sl, :], rb[av_sl, :])

        # Per-(lt, st) extra work schedule. KT chunk c+1 and V s-tiles
        # 4c+4..4c+7 are consumed at st = 4(c+1), so emit them during the
        # preceding four sts of lt0; QT for lt+1 late in each lt; the
        # out-projection of lt-1 spread through lt.
        jit = {}
        for c in range(3):
            for j in range(4):  # KT chunk c+1: ft=j//2, half=j%2
                jit.setdefault((0, 4 * c + j), []).append(
                    lambda c=c, j=j: emit_kt_half(c + 1, j // 2, j % 2))
        for st in range(12):
            jit.setdefault((0, st), []).append(
                lambda st=st: emit_v_st(st + 4))
        for lt in range(3):
            for j in range(4):
                jit.setdefault((lt, 10 + j), []).append(
                    lambda lt=lt, j=j: emit_qt_half(lt + 1, j // 2, j % 2))
        for lt in range(1, 4):
            for j, st in enumerate((4, 7, 10, 13)):
                jit.setdefault((lt, st), []).append(
                    lambda lt=lt, j=j: emit_outproj_tile(4 * (lt - 1) + j))

        # ---------------- prologue ----------------
        for j in range(4):
            emit_kt_half(0, j // 2, j % 2)
        for st in range(4):
            emit_v_st(st)
        for j in range(4):
            emit_qt_half(0, j // 2, j % 2)

        # ---------------- main loop ----------------
        for lt in range(LT):
            lsl = slice(lt * LTW, (lt + 1) * LTW)
            for h in range(HPC):
                Ts[h] = tp.tile([128, LTW], F32, tag=f"T{h}",
                                name=f"T{h}_{lt}")
            mk = mks[lt % 3]
            pending = []   # [(st, Es)]: AV lags scores by two s-tiles
            for st in range(ST):
                # prefetch next lt's masks in quarters spread across sts
                if lt < LT - 1 and st in (5, 7, 9, 11):
                    if st == 5:
                        mks[(lt + 1) % 3] = mpool.tile([128, ST, LTW], F16,
                                                       tag="mk",
                                                       name=f"mk{lt + 1}")
                    j = (st - 5) // 2
                    nc.sync.dma_start(
                        out=mks[(lt + 1) % 3][:, 4 * j:4 * j + 4, :],
                        in_=maskT[lt + 1, :, 4 * j:4 * j + 4, :])
                ssl = slice(st * STW, (st + 1) * STW)
                for fn in jit.get((lt, st), ()):
                    fn()
                Es = []
                for pair in range(2):
                    sc = scp.tile([128, 2, LTW], F32, tag="sc")
                    for i in range(2):
                        nc.tensor.matmul(
                            sc[:, i, :],
                            KT[64 * i:64 * (i + 1), pair, ssl],
                            QT[64 * i:64 * (i + 1), pair, lsl],
                            start=True, stop=True)
                    E = epool.tile([128, 2, LTW], F16, name=f"E{pair}")
                    nc.scalar.activation(E[:], sc[:], Exp)
                    nc.vector.tensor_tensor(
                        out=E[:], in0=E[:],
                        in1=mk[:, st, None, :].to_broadcast((128, 2, LTW)),
                        op=Mult)
                    Es.append(E)
                if len(pending) >= 2:
                    pst, pEs = pending.pop(0)
                    for pair in range(2):
                        for i in range(2):
                            h = 2 * pair + i
                            nc.tensor.matmul(Ts[h][:],
                                             Vaug[:, pst, h, :],
                                             pEs[pair][:, i, :],
                                             start=(pst == 0), stop=False)
                pending.append((st, Es))
            for pst, pEs in pending:
                for pair in range(2):
                    for i in range(2):
                        h = 2 * pair + i
                        nc.tensor.matmul(Ts[h][:], Vaug[:, pst, h, :],
                                         pEs[pair][:, i, :],
                                         start=(pst == 0),
                                         stop=(pst == ST - 1))
            for h in range(HPC):
                emit_normalize_h(lt, h)
            if lt == LT - 1:
                for lt8 in range(12, 16):
                    emit_outproj_tile(lt8)

    nc.compile()
    return nc


def _get_nc():
    global _CACHED_NC
    if _CACHED_NC is None:
        _CACHED_NC = _build()
    return _CACHED_NC


def _prep_core_inputs(c, x, mask, Wq, bq, Wk, bk, Wv, Wo):
    b, g = divmod(c, 4)
    cs = slice(g * FPC, (g + 1) * FPC)

    # xT[lt, p, kd, lw] = x[b][lt*512 + lw, kd*128 + p]
    xTc = np.ascontiguousarray(
        x[b].T.reshape(KD, 128, LT, LTW).transpose(2, 1, 0, 3)
    ).astype(np.float16)
    wq_c = np.ascontiguousarray(
        Wq[:, cs].reshape(KD, 128, FPC).transpose(1, 0, 2)).astype(np.float16)
    wk_c = np.ascontiguousarray(
        Wk[:, cs].reshape(KD, 128, FPC).transpose(1, 0, 2)).astype(np.float16)
    wv_c = np.ascontiguousarray(
        Wv[:, cs].reshape(KD, 128, FPC).transpose(1, 0, 2)).astype(np.float16)
    wo_c = np.ascontiguousarray(
        Wo[cs, :].reshape(2, 128, D).transpose(1, 0, 2)).astype(np.float16)
    bq_c = np.ascontiguousarray(
        (bq[cs] * 0.125).reshape(2, 128).T).astype(np.float32)
    # maskT[lt, p, st, lw] = mask[b][lt*512 + lw, st*128 + p]
    mT = mask[b].astype(np.float16).T  # [S, L]
    maskTc = np.ascontiguousarray(
        mT.reshape(ST, 128, LT, LTW).transpose(2, 1, 0, 3))
    return {"xT": xTc, "wq": wq_c, "wk": wk_c, "wv": wv_c, "wo": wo_c,
            "bq": bq_c, "maskT": maskTc}


def kernel(x, mask, Wq, bq, Wk, bk, Wv, bv, Wo, bo):
    x = np.asarray(x, np.float32)
    mask = np.asarray(mask)
    Wq, bq = np.asarray(Wq, np.float32), np.asarray(bq, np.float32)
    Wk, bk = np.asarray(Wk, np.float32), np.asarray(bk, np.float32)
    Wv, bv = np.asarray(Wv, np.float32), np.asarray(bv, np.float32)
    Wo, bo = np.asarray(Wo, np.float32), np.asarray(bo, np.float32)

    nc = _get_nc()
    in_maps = [_prep_core_inputs(c, x, mask, Wq, bq, Wk, bk, Wv, Wo)
               for c in range(NCORES)]
    res = run_bass_kernel_spmd(nc, in_maps, list(range(NCORES)))

    const_vec = (bv @ Wo + bo).astype(np.float32)  # A rows sum to 1
    outs = []
    for b in range(B):
        acc = np.zeros((L, D), np.float32)
        for g in range(4):
            part = res.results[4 * b + g]["out"]  # [128, 16, 1024] fp16
            acc += part.transpose(1, 0, 2).reshape(L, D).astype(np.float32)
        acc += const_vec
        outs.append(acc)
    return np.stack(outs)
